# revision 5
# baseline (speedup 1.0000x reference)
"""Trainium2 Bass kernel for MockTriangleMultiplication (outgoing triangle update).

Full-input contract: kernel(**inputs) takes the unsharded reference inputs and
returns the full [1, 512, 512, 128] output. Internally shards the first N (row)
axis of z/mask across 8 NeuronCores (sequence parallel); b rows are AllGathered
(FastFold-style dynamic-axial parallelism for the outgoing einsum).

The axon tunnel to the devices moves ~35 MB/s, so wall time is dominated by
host<->device bytes, not device compute. Wire-minimizing design:
  up:   z quantized to int8 with a global scale (LayerNorm is invariant to
        per-token affine maps, so the device consumes the integer values
        directly — no dequant needed), 32 MB total + mask + weights.
  down: delta = out - z as int8 with a per-token f32 scale (quantized on
        device), 33 MB total. Residual z + delta*scale is applied on host.
  The donated output buffers are created on-device (jnp.zeros inside the jit)
  instead of being shipped as host zeros, and the jitted runner is cached
  across calls.

Device pipeline per core (rows r in its 64-row shard):
  phase 1: z int8 -> bf16 -> LN -> transpose -> 4 projections -> sigmoid gates
           (+mask) -> a^T, b^T stored [c, row, col] in bf16
  AllGather b^T over 8 cores -> b_all [rank, c, k_loc, j]
  phase 2: per channel c: OUT_c[i_shard, j] = A_c[i_shard, :] @ B_c  (PSUM k-acc)
  phase 3: delta = OUT @ W_z + b_z; per-token abs-max -> int8 quantize

LayerNorm affine (ln_w, ln_b) is folded into the projection weights/biases on
the host, so the device does plain whitening only.
"""

import numpy as np
import ml_dtypes

import concourse.bass as bass
import concourse.bacc as bacc
import concourse.tile as tile
import concourse.mybir as mybir
import concourse.bass_utils as bass_utils
import concourse.masks as masks

F32 = mybir.dt.float32
BF16 = mybir.dt.bfloat16
I8 = mybir.dt.int8
AF = mybir.ActivationFunctionType
OP = mybir.AluOpType

R = 8          # cores
N = 512        # sequence
C = 128        # channels (c_z == c_hid)
SH = N // R    # rows per core
T4 = N // C    # 128-token tiles per row (4)
NQ = N // C    # k-chunks of 128 in the einsum
OCT = 8        # channels per phase-2 block

ZSCALE = 127.0 / 6.0   # global int8 scale for z (z ~ N(0,1))
QMAX = 126.0           # delta quant target (<=126 pre-round: no i8 overflow)

_CACHE = {}


def _phase1(tc, cst, z8_rows, a_loc, b_loc):
    nc = tc.nc
    with (
        tc.tile_pool(name="p1", bufs=3) as p1,
        tc.tile_pool(name="p1st", bufs=3) as p1st,
        tc.tile_pool(name="ps_zt", bufs=2, space="PSUM") as ps_zt,
        tc.tile_pool(name="ps_proj", bufs=1, space="PSUM") as ps_proj,
        tc.tile_pool(name="ps_mask", bufs=1, space="PSUM") as ps_mask,
    ):
        for r in range(SH):
            z8_sb = p1.tile([C, N], I8, tag="z8_sb")
            # [tok, (t, c)] <- z8_rows[r] viewed (t p) c -> p t c
            nc.gpsimd.dma_start(
                z8_sb[:].rearrange("p (t c) -> p t c", t=T4),
                z8_rows[r].rearrange("(t p) c -> p t c", p=C),
            )
            z_sb = p1.tile([C, N], BF16, tag="z_sb")
            nc.vector.tensor_copy(z_sb[:], z8_sb[:])
            mu4 = p1st.tile([C, T4], F32, tag="mu4")
            ssq4 = p1st.tile([C, T4], F32, tag="ssq4")
            sq_scr = p1st.tile([C, C], BF16, tag="sq_scr")
            for t in range(T4):
                zt = z_sb[:, t * C:(t + 1) * C]
                nc.vector.tensor_reduce(mu4[:, t:t + 1], zt,
                                        mybir.AxisListType.X, OP.add)
                nc.scalar.activation(sq_scr[:], zt, AF.Square,
                                     accum_out=ssq4[:, t:t + 1])
            nmu4 = p1st.tile([C, T4], F32, tag="nmu4")
            nc.vector.tensor_scalar_mul(nmu4[:], mu4[:], -1.0 / C)
            mu2 = p1st.tile([C, T4], F32, tag="mu2")
            nc.vector.tensor_tensor(mu2[:], nmu4[:], nmu4[:], OP.mult)
            var4 = p1st.tile([C, T4], F32, tag="var4")
            nc.vector.tensor_scalar_mul(var4[:], ssq4[:], 1.0 / C)
            var4b = p1st.tile([C, T4], F32, tag="var4b")
            nc.vector.tensor_tensor(var4b[:], var4[:], mu2[:], OP.subtract)
            std4 = p1st.tile([C, T4], F32, tag="std4")
            nc.scalar.activation(std4[:], var4b[:], AF.Sqrt,
                                 bias=cst['eps'][:])
            rstd4 = p1st.tile([C, T4], F32, tag="rstd4")
            nc.vector.reciprocal(rstd4[:], std4[:])

            zn_sb = p1.tile([C, N], BF16, tag="zn_sb")
            zT_ps = ps_zt.tile([C, N], BF16, tag="zT_ps")
            for t in range(T4):
                zt = z_sb[:, t * C:(t + 1) * C]
                znt = zn_sb[:, t * C:(t + 1) * C]
                nc.vector.tensor_scalar(
                    znt, zt, nmu4[:, t:t + 1], rstd4[:, t:t + 1],
                    OP.add, OP.mult)
                nc.tensor.transpose(zT_ps[:, t * C:(t + 1) * C], znt,
                                    cst['ident'][:])
            zT_sb = p1.tile([C, N], BF16, tag="zT_sb")
            nc.vector.tensor_copy(zT_sb[:], zT_ps[:])

            pap = ps_proj.tile([C, N], F32, tag="pap")
            pag = ps_proj.tile([C, N], F32, tag="pag")
            pbp = ps_proj.tile([C, N], F32, tag="pbp")
            pbg = ps_proj.tile([C, N], F32, tag="pbg")
            nc.tensor.matmul(pap[:], cst['wap'][:], zT_sb[:], start=True, stop=True)
            nc.tensor.matmul(pag[:], cst['wag'][:], zT_sb[:], start=True, stop=True)
            nc.tensor.matmul(pbp[:], cst['wbp'][:], zT_sb[:], start=True, stop=True)
            nc.tensor.matmul(pbg[:], cst['wbg'][:], zT_sb[:], start=True, stop=True)

            pa_sb = p1.tile([C, N], BF16, tag="pa_sb")
            pb_sb = p1.tile([C, N], BF16, tag="pb_sb")
            ga_sb = p1.tile([C, N], BF16, tag="ga_sb")
            gb_sb = p1.tile([C, N], BF16, tag="gb_sb")
            nc.vector.tensor_scalar_add(pa_sb[:], pap[:], cst['bap'][:])
            nc.scalar.activation(pb_sb[:], pbp[:], AF.Identity,
                                 bias=cst['bbp'][:])
            nc.scalar.activation(ga_sb[:], pag[:], AF.Sigmoid,
                                 bias=cst['bag'][:])
            nc.scalar.activation(gb_sb[:], pbg[:], AF.Sigmoid,
                                 bias=cst['bbg'][:])

            a1 = p1.tile([C, N], BF16, tag="a1")
            b1 = p1.tile([C, N], BF16, tag="b1")
            nc.vector.tensor_tensor(a1[:], pa_sb[:], ga_sb[:], OP.mult)
            nc.vector.tensor_tensor(b1[:], pb_sb[:], gb_sb[:], OP.mult)
            # mask row broadcast to 128 partitions via K=1 ones-matmul
            mask_ps = ps_mask.tile([C, N], F32, tag="mask_ps")
            nc.tensor.matmul(mask_ps[:], cst['ones1'][:],
                             cst['mask'][:, r * N:(r + 1) * N],
                             start=True, stop=True)
            mask_sb = p1.tile([C, N], BF16, tag="mask_sb")
            nc.scalar.copy(mask_sb[:], mask_ps[:])
            am = p1.tile([C, N], BF16, tag="am")
            bm = p1.tile([C, N], BF16, tag="bm")
            nc.vector.tensor_tensor(am[:], a1[:], mask_sb[:], OP.mult)
            nc.vector.tensor_tensor(bm[:], b1[:], mask_sb[:], OP.mult)
            nc.sync.dma_start(a_loc[:, r, :], am[:])
            nc.sync.dma_start(b_loc[:, r, :], bm[:])


def _phase2(tc, a_loc, b_all, o_mid):
    nc = tc.nc
    with (
        tc.tile_pool(name="p2a", bufs=2) as p2a,
        tc.tile_pool(name="p2b", bufs=2) as p2b,
        tc.tile_pool(name="p2o", bufs=3) as p2o,
        tc.tile_pool(name="ps_o", bufs=2, space="PSUM") as ps_o_pool,
    ):
        b_all_v = b_all[:].rearrange("(r c) k j -> r c k j", r=R)
        a_2d = a_loc[:].rearrange("c i k -> (c i) k")
        for oc in range(C // OCT):
            aT_t = []
            for q in range(NQ):
                at = p2a.tile([C, OCT * SH], BF16, tag=f"aT{q}")
                # src: a_loc[c-octet, :, k-chunk] as [(c i), k] 2D
                nc.sync.dma_start_transpose(
                    at[:],
                    a_2d[OCT * oc * SH:OCT * (oc + 1) * SH,
                         C * q:C * (q + 1)],
                )
                aT_t.append(at)
            RK = C // SH  # ranks per 128-row k-chunk
            b_t = []
            for q in range(NQ):
                bt = p2b.tile([C, OCT * N], BF16, tag=f"bT{q}")
                for rr in range(RK):
                    nc.sync.dma_start(
                        bt[rr * SH:(rr + 1) * SH, :].rearrange(
                            "k (c j) -> k c j", c=OCT),
                        b_all_v[RK * q + rr,
                                OCT * oc:OCT * (oc + 1), :, :].rearrange(
                            "c k j -> k c j"),
                    )
                b_t.append(bt)
            for ci in range(0, OCT, 2):
                o_sb = p2o.tile([SH, 2 * N], BF16, tag="o_sb")
                for cj in range(2):
                    ps_o = ps_o_pool.tile([SH, N], F32, tag="ps_o")
                    for q in range(NQ):
                        nc.tensor.matmul(
                            ps_o[:],
                            aT_t[q][:, (ci + cj) * SH:(ci + cj + 1) * SH],
                            b_t[q][:, (ci + cj) * N:(ci + cj + 1) * N],
                            start=(q == 0), stop=(q == NQ - 1))
                    nc.vector.tensor_copy(o_sb[:, cj * N:(cj + 1) * N],
                                          ps_o[:])
                c0 = OCT * oc + ci
                nc.sync.dma_start(
                    o_mid[c0:c0 + 2, :, :].rearrange("c k j -> k c j"),
                    o_sb[:].rearrange("k (c j) -> k c j", c=2))


def _phase3(tc, cst, o_mid, dq_rows, dsc_rows):
    nc = tc.nc
    with (
        tc.tile_pool(name="p3", bufs=3) as p3,
        tc.tile_pool(name="ps_f", bufs=4, space="PSUM") as ps_f_pool,
    ):
        for r in range(SH):
            oT_sb = p3.tile([C, N], BF16, tag="oT_sb")
            nc.sync.dma_start(oT_sb[:], o_mid[:, r, :])
            q_sb = p3.tile([C, N], I8, tag="q_sb")
            sc_sb = p3.tile([C, T4], F32, tag="sc_sb")
            for t in range(T4):
                # delta tile: [tok_p, out_chan] = o^T chunk @ W_z + b_z
                ps_f = ps_f_pool.tile([C, C], F32, tag="ps_f")
                nc.tensor.matmul(ps_f[:], oT_sb[:, t * C:(t + 1) * C],
                                 cst['wz'][:], start=True, stop=True)
                d_sb = p3.tile([C, C], F32, tag="d_sb")
                nc.vector.tensor_tensor(d_sb[:], ps_f[:], cst['bzbc'][:],
                                        OP.add)
                # per-token (partition) abs-max -> int8 quantize
                dab = p3.tile([C, C], F32, tag="dab")
                nc.scalar.activation(dab[:], d_sb[:], AF.Abs)
                amax = p3.tile([C, 1], F32, tag="amax")
                nc.vector.tensor_reduce(amax[:], dab[:],
                                        mybir.AxisListType.X, OP.max)
                amc = p3.tile([C, 1], F32, tag="amc")
                nc.vector.tensor_scalar_max(amc[:], amax[:], 1e-30)
                rcp = p3.tile([C, 1], F32, tag="rcp")
                nc.vector.reciprocal(rcp[:], amc[:])
                rsc = p3.tile([C, 1], F32, tag="rsc")
                nc.vector.tensor_scalar_mul(rsc[:], rcp[:], QMAX)
                nc.scalar.activation(q_sb[:, t * C:(t + 1) * C], d_sb[:],
                                     AF.Identity, scale=rsc[:])
                nc.vector.tensor_scalar_mul(sc_sb[:, t:t + 1], amc[:],
                                            1.0 / QMAX)
            nc.sync.dma_start(
                dq_rows[r].rearrange("(t p) c -> p t c", p=C),
                q_sb[:].rearrange("p (t c) -> p t c", t=T4))
            nc.sync.dma_start(
                dsc_rows[r].rearrange("t p -> p t"), sc_sb[:])


def build():
    if 'nc' in _CACHE:
        return _CACHE['nc']
    nc = bacc.Bacc("TRN2", target_bir_lowering=False, debug=False,
                   num_devices=R)

    z8_rows = nc.dram_tensor("z8_rows", [SH, N, C], I8, kind="ExternalInput")
    mask_rows = nc.dram_tensor("mask_rows", [SH, N], F32, kind="ExternalInput")
    w_in = {}
    for nm in ("w_ap", "w_ag", "w_bp", "w_bg", "w_z"):
        w_in[nm] = nc.dram_tensor(nm, [C, C], BF16, kind="ExternalInput")
    b_in = {}
    for nm in ("b_ap", "b_ag", "b_bp", "b_bg"):
        b_in[nm] = nc.dram_tensor(nm, [C, 1], F32, kind="ExternalInput")
    bz_bc = nc.dram_tensor("bz_bc", [C, C], F32, kind="ExternalInput")
    dq_rows = nc.dram_tensor("dq_rows", [SH, N, C], I8, kind="ExternalOutput")
    dsc_rows = nc.dram_tensor("dsc_rows", [SH, T4, C], F32,
                              kind="ExternalOutput")

    with tile.TileContext(nc) as tc:
        with (
            tc.tile_pool(name="consts", bufs=1) as cpool,
            tc.tile_pool(name="dram", bufs=1, space="DRAM") as dram,
        ):
            cst = {}
            ident = cpool.tile([C, C], BF16)
            masks.make_identity(nc, ident[:])
            cst['ident'] = ident
            for nm, key in (("w_ap", 'wap'), ("w_ag", 'wag'),
                            ("w_bp", 'wbp'), ("w_bg", 'wbg'), ("w_z", 'wz')):
                t = cpool.tile([C, C], BF16, tag=f"c_{key}")
                nc.sync.dma_start(t[:], w_in[nm][:])
                cst[key] = t
            for nm, key in (("b_ap", 'bap'), ("b_ag", 'bag'),
                            ("b_bp", 'bbp'), ("b_bg", 'bbg')):
                t = cpool.tile([C, 1], F32, tag=f"c_{key}")
                nc.sync.dma_start(t[:], b_in[nm][:])
                cst[key] = t
            bzbc = cpool.tile([C, C], F32)
            nc.sync.dma_start(bzbc[:], bz_bc[:])
            cst['bzbc'] = bzbc
            # whole mask shard on partition 0, bf16 (for K=1 broadcast matmuls)
            mask_p0 = cpool.tile([1, SH * N], BF16)
            nc.gpsimd.dma_start(mask_p0[:],
                                mask_rows[:].rearrange("r n -> (r n)")
                                .unsqueeze(0))
            cst['mask'] = mask_p0
            ones1 = cpool.tile([1, C], BF16)
            nc.vector.memset(ones1[:], 1.0)
            cst['ones1'] = ones1
            eps = cpool.tile([C, 1], F32)
            nc.vector.memset(eps[:], 1e-5)
            cst['eps'] = eps

            a_loc = dram.tile([C, SH, N], BF16)      # [c, i_loc, k]
            b_loc = dram.tile([C, SH, N], BF16)      # [c, k_loc, j]
            b_all = dram.tile([R * C, SH, N], BF16)  # [(rank c), k_loc, j]
            o_mid = dram.tile([C, SH, N], BF16)      # [c, i_loc, j]

            _phase1(tc, cst, z8_rows, a_loc, b_loc)
            nc.gpsimd.collective_compute(
                "AllGather", OP.bypass,
                replica_groups=[list(range(R))],
                ins=[b_loc[:].opt()],
                outs=[b_all[:].opt()],
            )
            _phase2(tc, a_loc, b_all, o_mid)
            _phase3(tc, cst, o_mid, dq_rows, dsc_rows)

    nc.compile()
    _CACHE['nc'] = nc
    return nc


def _get_runner():
    """Cached jitted SPMD runner (same mechanism run_bass_kernel_spmd uses
    under axon, hoisted so tracing/compilation happens once and the donated
    output buffers are created on-device instead of being uploaded)."""
    if 'runner' in _CACHE:
        return _CACHE['runner']
    import jax
    import jax.numpy as jnp
    from jax.sharding import Mesh, PartitionSpec
    from jax.experimental.shard_map import shard_map
    from concourse.bass2jax import (_bass_exec_p, partition_id_tensor,
                                    install_neuronx_cc_hook)

    nc = build()
    install_neuronx_cc_hook()
    partition_name = (nc.partition_id_tensor.name
                      if nc.partition_id_tensor else None)
    in_names, out_names, out_avals = [], [], []
    for alloc in nc.m.functions[0].allocations:
        if not isinstance(alloc, mybir.MemoryLocationSet):
            continue
        name = alloc.memorylocations[0].name
        if alloc.kind == "ExternalInput":
            if name != partition_name:
                in_names.append(name)
        elif alloc.kind == "ExternalOutput":
            out_names.append(name)
            out_avals.append(jax.core.ShapedArray(
                tuple(alloc.tensor_shape), mybir.dt.np(alloc.dtype)))
    all_names = in_names + out_names + (
        [partition_name] if partition_name else [])

    def _body(*args):
        operands = list(args)
        if partition_name is not None:
            operands.append(partition_id_tensor())
        outs = _bass_exec_p.bind(
            *operands, out_avals=tuple(out_avals), in_names=tuple(all_names),
            out_names=tuple(out_names),
            lowering_input_output_aliases=(),
            sim_require_finite=True, sim_require_nnan=True, nc=nc)
        return tuple(outs)

    devices = jax.devices()[:R]
    mesh = Mesh(np.asarray(devices), ("core",))
    n_args = len(in_names) + len(out_names)
    sharded = jax.jit(shard_map(
        _body, mesh=mesh,
        in_specs=(PartitionSpec("core"),) * n_args,
        out_specs=(PartitionSpec("core"),) * len(out_names),
        check_rep=False))
    # The donated "output" operands the bass_exec custom call expects are
    # materialized once ON-DEVICE (zero wire traffic) and reused every call.
    from jax.sharding import NamedSharding
    shardings = tuple(NamedSharding(mesh, PartitionSpec("core"))
                      for _ in out_avals)
    zeros_fn = jax.jit(
        lambda: tuple(jnp.zeros((R * a.shape[0],) + a.shape[1:], a.dtype)
                      for a in out_avals),
        out_shardings=shardings)
    zero_args = jax.block_until_ready(zeros_fn())
    _CACHE['runner'] = (sharded, in_names, out_names, zero_args)
    return _CACHE['runner']


def _host_fns():
    if 'host' in _CACHE:
        return _CACHE['host']
    import jax
    import jax.numpy as jnp
    cpu = jax.devices("cpu")[0]

    def _quant(z):
        q = jnp.clip(jnp.round(z * ZSCALE), -127.0, 127.0)
        return q.astype(jnp.int8)

    def _post(z, dq, sc):
        return z + dq.astype(jnp.float32) * sc[..., None]

    quant = jax.jit(_quant, device=cpu)
    post = jax.jit(_post, device=cpu)
    _CACHE['host'] = (quant, post, cpu)
    return _CACHE['host']


def kernel(z, mask, ln_w, ln_b, W_ap, b_ap, W_ag, b_ag, W_bp, b_bp,
           W_bg, b_bg, W_z, b_z):
    import jax
    z = np.asarray(z, dtype=np.float32).reshape(N, N, C)
    mask = np.asarray(mask, dtype=np.float32).reshape(N, N)
    ln_w = np.asarray(ln_w, np.float32)
    ln_b = np.asarray(ln_b, np.float32)
    bf = ml_dtypes.bfloat16

    def fold_w(W):
        return np.tile((ln_w[:, None] * np.asarray(W, np.float32))
                       .astype(bf), (R, 1))

    def fold_b(b, W):
        return np.tile(
            (np.asarray(b, np.float32) + ln_b @ np.asarray(W, np.float32))
            .reshape(C, 1), (R, 1))

    quant, post, cpu = _host_fns()
    z8 = np.asarray(quant(z))

    global_ins = dict(
        z8_rows=z8,
        mask_rows=mask,
        w_ap=fold_w(W_ap), w_ag=fold_w(W_ag),
        w_bp=fold_w(W_bp), w_bg=fold_w(W_bg),
        b_ap=fold_b(b_ap, W_ap), b_ag=fold_b(b_ag, W_ag),
        b_bp=fold_b(b_bp, W_bp), b_bg=fold_b(b_bg, W_bg),
        w_z=np.tile(np.asarray(W_z, np.float32).astype(bf), (R, 1)),
        bz_bc=np.tile(np.broadcast_to(
            np.asarray(b_z, np.float32), (C, C)), (R, 1)),
    )

    sharded, in_names, out_names, zero_args = _get_runner()
    outs = sharded(*[global_ins[n] for n in in_names], *zero_args)
    res = {n: outs[i] for i, n in enumerate(out_names)}
    dq = np.asarray(res['dq_rows'])            # [N, N, C] int8
    sc = np.asarray(res['dsc_rows'])           # [N, T4, C] f32
    out = np.asarray(post(z, dq, sc.reshape(N, N)))
    return out.reshape(1, N, N, C)


# revision 8
# speedup vs baseline: 1.0399x; 1.0399x over previous
"""Trainium2 Bass kernel for MockTriangleMultiplication (outgoing triangle update).

Full-input contract: kernel(**inputs) takes the unsharded reference inputs and
returns the full [1, 512, 512, 128] output. Internally shards the first N (row)
axis of z/mask across 8 NeuronCores (sequence parallel); b rows are AllGathered
(FastFold-style dynamic-axial parallelism for the outgoing einsum).

The axon tunnel to the devices moves ~35 MB/s, so wall time is dominated by
host<->device bytes, not device compute. Wire-minimizing design:
  up:   z quantized to int8 with a global scale (LayerNorm is invariant to
        per-token affine maps, so the device consumes the integer values
        directly — no dequant needed), 32 MB total + mask + weights.
  down: delta = out - z as int8 with a per-token f32 scale (quantized on
        device), 33 MB total. Residual z + delta*scale is applied on host.
  The donated output buffers are created on-device (jnp.zeros inside the jit)
  instead of being shipped as host zeros, and the jitted runner is cached
  across calls.

Device pipeline per core (rows r in its 64-row shard):
  phase 1: z int8 -> bf16 -> LN -> transpose -> 4 projections -> sigmoid gates
           (+mask) -> a^T, b^T stored [c, row, col] in bf16
  AllGather b^T over 8 cores -> b_all [rank, c, k_loc, j]
  phase 2: per channel c: OUT_c[i_shard, j] = A_c[i_shard, :] @ B_c  (PSUM k-acc)
  phase 3: delta = OUT @ W_z + b_z; per-token abs-max -> int8 quantize

LayerNorm affine (ln_w, ln_b) is folded into the projection weights/biases on
the host, so the device does plain whitening only.
"""

import numpy as np
import ml_dtypes

import concourse.bass as bass
import concourse.bacc as bacc
import concourse.tile as tile
import concourse.mybir as mybir
import concourse.bass_utils as bass_utils
import concourse.masks as masks

F32 = mybir.dt.float32
BF16 = mybir.dt.bfloat16
I8 = mybir.dt.int8
AF = mybir.ActivationFunctionType
OP = mybir.AluOpType

R = 8          # cores
N = 512        # sequence
C = 128        # channels (c_z == c_hid)
SH = N // R    # rows per core
T4 = N // C    # 128-token tiles per row (4)
NQ = N // C    # k-chunks of 128 in the einsum
OCT = 8        # channels per phase-2 block

ZSCALE = 127.0 / 6.0   # global int8 scale for z (z ~ N(0,1))
QMAX = 126.0           # delta quant target (<=126 pre-round: no i8 overflow)

_CACHE = {}


def _phase1(tc, cst, z8_rows, a_loc, b_loc):
    nc = tc.nc
    with (
        tc.tile_pool(name="p1", bufs=3) as p1,
        tc.tile_pool(name="p1st", bufs=3) as p1st,
        tc.tile_pool(name="ps_zt", bufs=2, space="PSUM") as ps_zt,
        tc.tile_pool(name="ps_proj", bufs=1, space="PSUM") as ps_proj,
        tc.tile_pool(name="ps_mask", bufs=1, space="PSUM") as ps_mask,
    ):
        for r in range(SH):
            z8_sb = p1.tile([C, N], I8, tag="z8_sb")
            # [tok, (t, c)] <- z8_rows[r] viewed (t p) c -> p t c
            nc.gpsimd.dma_start(
                z8_sb[:].rearrange("p (t c) -> p t c", t=T4),
                z8_rows[r].rearrange("(t p) c -> p t c", p=C),
            )
            z_sb = p1.tile([C, N], BF16, tag="z_sb")
            nc.vector.tensor_copy(z_sb[:], z8_sb[:])
            mu4 = p1st.tile([C, T4], F32, tag="mu4")
            ssq4 = p1st.tile([C, T4], F32, tag="ssq4")
            sq_scr = p1st.tile([C, C], BF16, tag="sq_scr")
            for t in range(T4):
                zt = z_sb[:, t * C:(t + 1) * C]
                nc.vector.tensor_reduce(mu4[:, t:t + 1], zt,
                                        mybir.AxisListType.X, OP.add)
                nc.scalar.activation(sq_scr[:], zt, AF.Square,
                                     accum_out=ssq4[:, t:t + 1])
            nmu4 = p1st.tile([C, T4], F32, tag="nmu4")
            nc.vector.tensor_scalar_mul(nmu4[:], mu4[:], -1.0 / C)
            mu2 = p1st.tile([C, T4], F32, tag="mu2")
            nc.vector.tensor_tensor(mu2[:], nmu4[:], nmu4[:], OP.mult)
            var4 = p1st.tile([C, T4], F32, tag="var4")
            nc.vector.tensor_scalar_mul(var4[:], ssq4[:], 1.0 / C)
            var4b = p1st.tile([C, T4], F32, tag="var4b")
            nc.vector.tensor_tensor(var4b[:], var4[:], mu2[:], OP.subtract)
            std4 = p1st.tile([C, T4], F32, tag="std4")
            nc.scalar.activation(std4[:], var4b[:], AF.Sqrt,
                                 bias=cst['eps'][:])
            rstd4 = p1st.tile([C, T4], F32, tag="rstd4")
            nc.vector.reciprocal(rstd4[:], std4[:])

            zn_sb = p1.tile([C, N], BF16, tag="zn_sb")
            zT_ps = ps_zt.tile([C, N], BF16, tag="zT_ps")
            for t in range(T4):
                zt = z_sb[:, t * C:(t + 1) * C]
                znt = zn_sb[:, t * C:(t + 1) * C]
                nc.vector.tensor_scalar(
                    znt, zt, nmu4[:, t:t + 1], rstd4[:, t:t + 1],
                    OP.add, OP.mult)
                nc.tensor.transpose(zT_ps[:, t * C:(t + 1) * C], znt,
                                    cst['ident'][:])
            zT_sb = p1.tile([C, N], BF16, tag="zT_sb")
            nc.vector.tensor_copy(zT_sb[:], zT_ps[:])

            pap = ps_proj.tile([C, N], F32, tag="pap")
            pag = ps_proj.tile([C, N], F32, tag="pag")
            pbp = ps_proj.tile([C, N], F32, tag="pbp")
            pbg = ps_proj.tile([C, N], F32, tag="pbg")
            nc.tensor.matmul(pap[:], cst['wap'][:], zT_sb[:], start=True, stop=True)
            nc.tensor.matmul(pag[:], cst['wag'][:], zT_sb[:], start=True, stop=True)
            nc.tensor.matmul(pbp[:], cst['wbp'][:], zT_sb[:], start=True, stop=True)
            nc.tensor.matmul(pbg[:], cst['wbg'][:], zT_sb[:], start=True, stop=True)

            pa_sb = p1.tile([C, N], BF16, tag="pa_sb")
            pb_sb = p1.tile([C, N], BF16, tag="pb_sb")
            ga_sb = p1.tile([C, N], BF16, tag="ga_sb")
            gb_sb = p1.tile([C, N], BF16, tag="gb_sb")
            nc.vector.tensor_scalar_add(pa_sb[:], pap[:], cst['bap'][:])
            nc.scalar.activation(pb_sb[:], pbp[:], AF.Identity,
                                 bias=cst['bbp'][:])
            nc.scalar.activation(ga_sb[:], pag[:], AF.Sigmoid,
                                 bias=cst['bag'][:])
            nc.scalar.activation(gb_sb[:], pbg[:], AF.Sigmoid,
                                 bias=cst['bbg'][:])

            a1 = p1.tile([C, N], BF16, tag="a1")
            b1 = p1.tile([C, N], BF16, tag="b1")
            nc.vector.tensor_tensor(a1[:], pa_sb[:], ga_sb[:], OP.mult)
            nc.vector.tensor_tensor(b1[:], pb_sb[:], gb_sb[:], OP.mult)
            # mask row broadcast to 128 partitions via K=1 ones-matmul
            mask_ps = ps_mask.tile([C, N], F32, tag="mask_ps")
            nc.tensor.matmul(mask_ps[:], cst['ones1'][:],
                             cst['mask'][:, r * N:(r + 1) * N],
                             start=True, stop=True)
            mask_sb = p1.tile([C, N], BF16, tag="mask_sb")
            nc.scalar.copy(mask_sb[:], mask_ps[:])
            am = p1.tile([C, N], BF16, tag="am")
            bm = p1.tile([C, N], BF16, tag="bm")
            nc.vector.tensor_tensor(am[:], a1[:], mask_sb[:], OP.mult)
            nc.vector.tensor_tensor(bm[:], b1[:], mask_sb[:], OP.mult)
            nc.sync.dma_start(a_loc[:, r, :], am[:])
            nc.sync.dma_start(b_loc[:, r, :], bm[:])


def _phase2(tc, a_loc, b_all, o_mid):
    nc = tc.nc
    with (
        tc.tile_pool(name="p2a", bufs=2) as p2a,
        tc.tile_pool(name="p2b", bufs=2) as p2b,
        tc.tile_pool(name="p2o", bufs=3) as p2o,
        tc.tile_pool(name="ps_o", bufs=2, space="PSUM") as ps_o_pool,
    ):
        b_all_v = b_all[:].rearrange("(r c) k j -> r c k j", r=R)
        a_2d = a_loc[:].rearrange("c i k -> (c i) k")
        for oc in range(C // OCT):
            aT_t = []
            for q in range(NQ):
                at = p2a.tile([C, OCT * SH], BF16, tag=f"aT{q}")
                # src: a_loc[c-octet, :, k-chunk] as [(c i), k] 2D
                nc.sync.dma_start_transpose(
                    at[:],
                    a_2d[OCT * oc * SH:OCT * (oc + 1) * SH,
                         C * q:C * (q + 1)],
                )
                aT_t.append(at)
            RK = C // SH  # ranks per 128-row k-chunk
            b_t = []
            for q in range(NQ):
                bt = p2b.tile([C, OCT * N], BF16, tag=f"bT{q}")
                for rr in range(RK):
                    nc.sync.dma_start(
                        bt[rr * SH:(rr + 1) * SH, :].rearrange(
                            "k (c j) -> k c j", c=OCT),
                        b_all_v[RK * q + rr,
                                OCT * oc:OCT * (oc + 1), :, :].rearrange(
                            "c k j -> k c j"),
                    )
                b_t.append(bt)
            for ci in range(0, OCT, 2):
                o_sb = p2o.tile([SH, 2 * N], BF16, tag="o_sb")
                for cj in range(2):
                    ps_o = ps_o_pool.tile([SH, N], F32, tag="ps_o")
                    for q in range(NQ):
                        nc.tensor.matmul(
                            ps_o[:],
                            aT_t[q][:, (ci + cj) * SH:(ci + cj + 1) * SH],
                            b_t[q][:, (ci + cj) * N:(ci + cj + 1) * N],
                            start=(q == 0), stop=(q == NQ - 1))
                    nc.vector.tensor_copy(o_sb[:, cj * N:(cj + 1) * N],
                                          ps_o[:])
                c0 = OCT * oc + ci
                nc.sync.dma_start(
                    o_mid[c0:c0 + 2, :, :].rearrange("c k j -> k c j"),
                    o_sb[:].rearrange("k (c j) -> k c j", c=2))


def _phase3(tc, cst, o_mid, dq_rows, dsc_rows):
    nc = tc.nc
    with (
        tc.tile_pool(name="p3", bufs=3) as p3,
        tc.tile_pool(name="ps_f", bufs=4, space="PSUM") as ps_f_pool,
    ):
        for r in range(SH):
            oT_sb = p3.tile([C, N], BF16, tag="oT_sb")
            nc.sync.dma_start(oT_sb[:], o_mid[:, r, :])
            q_sb = p3.tile([C, N], I8, tag="q_sb")
            sc_sb = p3.tile([C, T4], F32, tag="sc_sb")
            for t in range(T4):
                # delta tile: [tok_p, out_chan] = o^T chunk @ W_z + b_z
                ps_f = ps_f_pool.tile([C, C], F32, tag="ps_f")
                nc.tensor.matmul(ps_f[:], oT_sb[:, t * C:(t + 1) * C],
                                 cst['wz'][:], start=True, stop=True)
                d_sb = p3.tile([C, C], F32, tag="d_sb")
                nc.vector.tensor_tensor(d_sb[:], ps_f[:], cst['bzbc'][:],
                                        OP.add)
                # per-token (partition) abs-max -> int8 quantize
                dab = p3.tile([C, C], F32, tag="dab")
                nc.scalar.activation(dab[:], d_sb[:], AF.Abs)
                amax = p3.tile([C, 1], F32, tag="amax")
                nc.vector.tensor_reduce(amax[:], dab[:],
                                        mybir.AxisListType.X, OP.max)
                amc = p3.tile([C, 1], F32, tag="amc")
                nc.vector.tensor_scalar_max(amc[:], amax[:], 1e-30)
                rcp = p3.tile([C, 1], F32, tag="rcp")
                nc.vector.reciprocal(rcp[:], amc[:])
                rsc = p3.tile([C, 1], F32, tag="rsc")
                nc.vector.tensor_scalar_mul(rsc[:], rcp[:], QMAX)
                nc.scalar.activation(q_sb[:, t * C:(t + 1) * C], d_sb[:],
                                     AF.Identity, scale=rsc[:])
                nc.vector.tensor_scalar_mul(sc_sb[:, t:t + 1], amc[:],
                                            1.0 / QMAX)
            nc.sync.dma_start(
                dq_rows[r].rearrange("(t p) c -> p t c", p=C),
                q_sb[:].rearrange("p (t c) -> p t c", t=T4))
            nc.sync.dma_start(
                dsc_rows[r].rearrange("t p -> p t"), sc_sb[:])


def build():
    if 'nc' in _CACHE:
        return _CACHE['nc']
    nc = bacc.Bacc("TRN2", target_bir_lowering=False, debug=False,
                   num_devices=R)

    z8_rows = nc.dram_tensor("z8_rows", [SH, N, C], I8, kind="ExternalInput")
    mask_rows = nc.dram_tensor("mask_rows", [SH, N], F32, kind="ExternalInput")
    w_in = {}
    for nm in ("w_ap", "w_ag", "w_bp", "w_bg", "w_z"):
        w_in[nm] = nc.dram_tensor(nm, [C, C], BF16, kind="ExternalInput")
    b_in = {}
    for nm in ("b_ap", "b_ag", "b_bp", "b_bg"):
        b_in[nm] = nc.dram_tensor(nm, [C, 1], F32, kind="ExternalInput")
    bz_bc = nc.dram_tensor("bz_bc", [C, C], F32, kind="ExternalInput")
    dq_rows = nc.dram_tensor("dq_rows", [SH, N, C], I8, kind="ExternalOutput")
    dsc_rows = nc.dram_tensor("dsc_rows", [SH, T4, C], F32,
                              kind="ExternalOutput")

    with tile.TileContext(nc) as tc:
        with (
            tc.tile_pool(name="consts", bufs=1) as cpool,
            tc.tile_pool(name="dram", bufs=1, space="DRAM") as dram,
        ):
            cst = {}
            ident = cpool.tile([C, C], BF16)
            masks.make_identity(nc, ident[:])
            cst['ident'] = ident
            for nm, key in (("w_ap", 'wap'), ("w_ag", 'wag'),
                            ("w_bp", 'wbp'), ("w_bg", 'wbg'), ("w_z", 'wz')):
                t = cpool.tile([C, C], BF16, tag=f"c_{key}")
                nc.sync.dma_start(t[:], w_in[nm][:])
                cst[key] = t
            for nm, key in (("b_ap", 'bap'), ("b_ag", 'bag'),
                            ("b_bp", 'bbp'), ("b_bg", 'bbg')):
                t = cpool.tile([C, 1], F32, tag=f"c_{key}")
                nc.sync.dma_start(t[:], b_in[nm][:])
                cst[key] = t
            bzbc = cpool.tile([C, C], F32)
            nc.sync.dma_start(bzbc[:], bz_bc[:])
            cst['bzbc'] = bzbc
            # whole mask shard on partition 0, bf16 (for K=1 broadcast matmuls)
            mask_p0 = cpool.tile([1, SH * N], BF16)
            nc.gpsimd.dma_start(mask_p0[:],
                                mask_rows[:].rearrange("r n -> (r n)")
                                .unsqueeze(0))
            cst['mask'] = mask_p0
            ones1 = cpool.tile([1, C], BF16)
            nc.vector.memset(ones1[:], 1.0)
            cst['ones1'] = ones1
            eps = cpool.tile([C, 1], F32)
            nc.vector.memset(eps[:], 1e-5)
            cst['eps'] = eps

            a_loc = dram.tile([C, SH, N], BF16)      # [c, i_loc, k]
            b_loc = dram.tile([C, SH, N], BF16)      # [c, k_loc, j]
            b_all = dram.tile([R * C, SH, N], BF16)  # [(rank c), k_loc, j]
            o_mid = dram.tile([C, SH, N], BF16)      # [c, i_loc, j]

            _phase1(tc, cst, z8_rows, a_loc, b_loc)
            nc.gpsimd.collective_compute(
                "AllGather", OP.bypass,
                replica_groups=[list(range(R))],
                ins=[b_loc[:].opt()],
                outs=[b_all[:].opt()],
            )
            _phase2(tc, a_loc, b_all, o_mid)
            _phase3(tc, cst, o_mid, dq_rows, dsc_rows)

    nc.compile()
    _CACHE['nc'] = nc
    return nc


def _get_runner():
    """Cached jitted SPMD runner (same mechanism run_bass_kernel_spmd uses
    under axon, hoisted so tracing/compilation happens once and the donated
    output buffers are created on-device instead of being uploaded)."""
    if 'runner' in _CACHE:
        return _CACHE['runner']
    import jax
    import jax.numpy as jnp
    from jax.sharding import Mesh, PartitionSpec
    from jax.experimental.shard_map import shard_map
    from concourse.bass2jax import (_bass_exec_p, partition_id_tensor,
                                    install_neuronx_cc_hook)

    nc = build()
    install_neuronx_cc_hook()
    partition_name = (nc.partition_id_tensor.name
                      if nc.partition_id_tensor else None)
    in_names, out_names, out_avals = [], [], []
    for alloc in nc.m.functions[0].allocations:
        if not isinstance(alloc, mybir.MemoryLocationSet):
            continue
        name = alloc.memorylocations[0].name
        if alloc.kind == "ExternalInput":
            if name != partition_name:
                in_names.append(name)
        elif alloc.kind == "ExternalOutput":
            out_names.append(name)
            out_avals.append(jax.core.ShapedArray(
                tuple(alloc.tensor_shape), mybir.dt.np(alloc.dtype)))
    all_names = in_names + out_names + (
        [partition_name] if partition_name else [])

    def _body(*args):
        operands = list(args)
        if partition_name is not None:
            operands.append(partition_id_tensor())
        outs = _bass_exec_p.bind(
            *operands, out_avals=tuple(out_avals), in_names=tuple(all_names),
            out_names=tuple(out_names),
            lowering_input_output_aliases=(),
            sim_require_finite=True, sim_require_nnan=True, nc=nc)
        return tuple(outs)

    devices = jax.devices()[:R]
    mesh = Mesh(np.asarray(devices), ("core",))
    n_args = len(in_names) + len(out_names)
    sharded = jax.jit(shard_map(
        _body, mesh=mesh,
        in_specs=(PartitionSpec("core"),) * n_args,
        out_specs=(PartitionSpec("core"),) * len(out_names),
        check_rep=False))
    # The donated "output" operands the bass_exec custom call expects are
    # materialized once ON-DEVICE (zero wire traffic) and reused every call.
    from jax.sharding import NamedSharding
    shardings = tuple(NamedSharding(mesh, PartitionSpec("core"))
                      for _ in out_avals)
    zeros_fn = jax.jit(
        lambda: tuple(jnp.zeros((R * a.shape[0],) + a.shape[1:], a.dtype)
                      for a in out_avals),
        out_shardings=shardings)
    zero_args = jax.block_until_ready(zeros_fn())
    _CACHE['runner'] = (sharded, in_names, out_names, zero_args)
    return _CACHE['runner']


def _host_fns():
    if 'host' in _CACHE:
        return _CACHE['host']
    import jax
    import jax.numpy as jnp
    cpu = jax.devices("cpu")[0]

    def _quant(z):
        q = jnp.clip(jnp.round(z * ZSCALE), -127.0, 127.0)
        return q.astype(jnp.int8)

    def _post(z, dq, sc):
        return z + dq.astype(jnp.float32) * sc[..., None]

    quant = jax.jit(_quant, device=cpu)
    post = jax.jit(_post, device=cpu)
    _CACHE['host'] = (quant, post, cpu)
    return _CACHE['host']


def kernel(z, mask, ln_w, ln_b, W_ap, b_ap, W_ag, b_ag, W_bp, b_bp,
           W_bg, b_bg, W_z, b_z):
    import jax
    import os, time
    _dbg = os.environ.get("K_TIMING") == "1"
    _t = time.time
    t0 = _t()
    z = np.asarray(z, dtype=np.float32).reshape(N, N, C)
    mask = np.asarray(mask, dtype=np.float32).reshape(N, N)
    ln_w = np.asarray(ln_w, np.float32)
    ln_b = np.asarray(ln_b, np.float32)
    bf = ml_dtypes.bfloat16

    def fold_w(W):
        return np.tile((ln_w[:, None] * np.asarray(W, np.float32))
                       .astype(bf), (R, 1))

    def fold_b(b, W):
        return np.tile(
            (np.asarray(b, np.float32) + ln_b @ np.asarray(W, np.float32))
            .reshape(C, 1), (R, 1))

    quant, post, cpu = _host_fns()
    z8 = np.asarray(quant(z))
    if _dbg:
        print(f"[t] quant: {_t()-t0:.3f}"); t0 = _t()

    global_ins = dict(
        z8_rows=z8,
        mask_rows=mask,
        w_ap=fold_w(W_ap), w_ag=fold_w(W_ag),
        w_bp=fold_w(W_bp), w_bg=fold_w(W_bg),
        b_ap=fold_b(b_ap, W_ap), b_ag=fold_b(b_ag, W_ag),
        b_bp=fold_b(b_bp, W_bp), b_bg=fold_b(b_bg, W_bg),
        w_z=np.tile(np.asarray(W_z, np.float32).astype(bf), (R, 1)),
        bz_bc=np.tile(np.broadcast_to(
            np.asarray(b_z, np.float32), (C, C)), (R, 1)),
    )

    sharded, in_names, out_names, zero_args = _get_runner()
    if _dbg:
        print(f"[t] prep/runner: {_t()-t0:.3f}"); t0 = _t()
    outs = sharded(*[global_ins[n] for n in in_names], *zero_args)
    if _dbg:
        import jax as _jax
        _jax.block_until_ready(outs)
        print(f"[t] dispatch+exec: {_t()-t0:.3f}"); t0 = _t()
    res = {n: outs[i] for i, n in enumerate(out_names)}
    dq = np.asarray(res['dq_rows'])            # [N, N, C] int8
    sc = np.asarray(res['dsc_rows'])           # [N, T4, C] f32
    if _dbg:
        print(f"[t] fetch: {_t()-t0:.3f}"); t0 = _t()
    out = np.asarray(post(z, dq, sc.reshape(N, N)))
    if _dbg:
        print(f"[t] post: {_t()-t0:.3f}")
    return out.reshape(1, N, N, C)
